# revision 3
# baseline (speedup 1.0000x reference)
"""Trainium2 Bass kernel for the MFCA channel-attention module.

  q = x_RGB.reshape(B, C, N); k = v = x.reshape(B, C, N)
  energy    = q @ k^T                          (B, C, C)
  attention = softmax(max(energy, -1) - energy)   over last axis
  out       = delta * (attention @ v) + x

Numerically, softmax(max - energy) == softmax(-energy), and the stable
form is p = exp(min_row(energy) - energy), attention = p / rowsum(p).

Sharding: data-parallel over batch B=16 across 8 NeuronCores (2 per core).
Each core computes its 2 batches fully: two bf16 matmul phases
(C x C x N and C x C x N), softmax on-chip, fp32 residual add.

Layout strategy per batch (C=512, N=4096):
  - x loaded fp32 (residual add) -> cast bf16 (V and K source)
  - x_RGB loaded via SWDGE cast-DMA straight to bf16 (Q source)
  - Q^T, K^T ([N, C] layout needed for the energy matmul, which
    contracts over N) produced with the DMA-transpose xbar (bf16)
  - energy accumulated in PSUM over 32 n-chunks; row-min + exp
    (with fused row-sum) on DVE/ACT; P^T via DMA-transpose
  - attention @ v accumulated in PSUM over 4 j-chunks; epilogue
    scales by delta/Z per row (ACT) and adds fp32 x (DVE)
"""

import numpy as np

import concourse.bass as bass
import concourse.tile as tile
from concourse import bacc, mybir
from concourse.bass_utils import run_bass_kernel_spmd

N_CORES = 8
B, C, H, W = 16, 512, 64, 64
N = H * W  # 4096
BS = B // N_CORES  # batches per core

F32 = mybir.dt.float32
BF16 = mybir.dt.bfloat16


def build_nc(bs=BS, c=C, n=N):
    """Build the single-core Bass program (SPMD across cores)."""
    nc = bacc.Bacc(None, target_bir_lowering=False, debug=False)

    x_d = nc.dram_tensor("x", [bs, c, n], F32, kind="ExternalInput")
    q_d = nc.dram_tensor("x_RGB", [bs, c, n], F32, kind="ExternalInput")
    d_d = nc.dram_tensor("delta", [128, 1], F32, kind="ExternalInput")
    o_d = nc.dram_tensor("out", [bs, c, n], F32, kind="ExternalOutput")

    nct = c // 128  # channel chunks (i-tiles / j-chunks)
    nnt = n // 128  # n-chunks for the energy contraction
    nnb = n // 512  # n-blocks for the output matmul

    from contextlib import ExitStack

    with tile.TileContext(nc) as tc, ExitStack() as ctx:
        px32 = ctx.enter_context(tc.tile_pool(name="px32", bufs=4))
        pxb = ctx.enter_context(tc.tile_pool(name="pxb", bufs=5))
        pqb = ctx.enter_context(tc.tile_pool(name="pqb", bufs=2))
        pxt = ctx.enter_context(tc.tile_pool(name="pxt", bufs=1))
        pqt = ctx.enter_context(tc.tile_pool(name="pqt", bufs=1))
        pp = ctx.enter_context(tc.tile_pool(name="pp", bufs=3))
        ppt = ctx.enter_context(tc.tile_pool(name="ppt", bufs=3))
        pout = ctx.enter_context(tc.tile_pool(name="pout", bufs=3))
        psml = ctx.enter_context(tc.tile_pool(name="psml", bufs=16))
        pone = ctx.enter_context(tc.tile_pool(name="pone", bufs=1))
        pe_pool = ctx.enter_context(tc.tile_pool(name="pe", bufs=2, space="PSUM"))
        pu_pool = ctx.enter_context(tc.tile_pool(name="pu", bufs=4, space="PSUM"))

        delta_sb = pone.tile([128, 1], F32)
        nc.sync.dma_start(out=delta_sb[:], in_=d_d[:])

        def emit_loads(b):
            x32s, xbs = [], []
            xt = pxt.tile([128, nnt, c], BF16)  # K^T: [n-part, nt, c]
            qt = pqt.tile([128, nnt, c], BF16)  # Q^T
            for k in range(nct):
                x32 = px32.tile([128, n], F32)
                nc.sync.dma_start(out=x32[:], in_=x_d[b, 128 * k : 128 * (k + 1), :])
                qb = pqb.tile([128, n], BF16)
                nc.gpsimd.dma_start(out=qb[:], in_=q_d[b, 128 * k : 128 * (k + 1), :])
                xb = pxb.tile([128, n], BF16)
                nc.vector.tensor_copy(out=xb[:], in_=x32[:])
                nc.scalar.dma_start(
                    out=xt[:, :, 128 * k : 128 * (k + 1)], in_=xb[:], transpose=True
                )
                nc.scalar.dma_start(
                    out=qt[:, :, 128 * k : 128 * (k + 1)], in_=qb[:], transpose=True
                )
                x32s.append(x32)
                xbs.append(xb)
            return x32s, xbs, xt, qt

        def emit_mm1_softmax(i, xt, qt):
            e = pe_pool.tile([128, c], F32)
            for nt in range(nnt):
                nc.tensor.matmul(
                    e[:],
                    qt[:, nt, 128 * i : 128 * (i + 1)],
                    xt[:, nt, :],
                    start=(nt == 0),
                    stop=(nt == nnt - 1),
                )
            m = psml.tile([128, 1], F32)
            nc.vector.tensor_reduce(
                m[:], e[:], axis=mybir.AxisListType.X, op=mybir.AluOpType.min
            )
            p_t = pp.tile([128, c], BF16)
            z = psml.tile([128, 1], F32)
            nc.scalar.activation(
                out=p_t[:],
                in_=e[:],
                func=mybir.ActivationFunctionType.Exp,
                bias=m[:],
                scale=-1.0,
                accum_out=z[:],
            )
            zi = psml.tile([128, 1], F32)
            nc.vector.reciprocal(zi[:], z[:])
            s = psml.tile([128, 1], F32)
            nc.vector.tensor_scalar_mul(s[:], zi[:], delta_sb[:])  # delta / Z
            pt = ppt.tile([128, nct, 128], BF16)  # P^T: [j-part, jt, i]
            nc.scalar.dma_start(out=pt[:], in_=p_t[:], transpose=True)
            return s, pt

        def emit_mm2_epi(b, i, s, pt, xbs, x32s):
            for nb in range(nnb):
                u = pu_pool.tile([128, 512], F32)
                for jt in range(nct):
                    nc.tensor.matmul(
                        u[:],
                        pt[:, jt, :],
                        xbs[jt][:, 512 * nb : 512 * (nb + 1)],
                        start=(jt == 0),
                        stop=(jt == nct - 1),
                    )
                t = pout.tile([128, 512], F32)
                nc.scalar.mul(t[:], u[:], s[:])  # (delta/Z_i) * U, per-partition
                o = pout.tile([128, 512], F32)
                nc.vector.tensor_add(o[:], t[:], x32s[i][:, 512 * nb : 512 * (nb + 1)])
                nc.sync.dma_start(
                    out=o_d[b, 128 * i : 128 * (i + 1), 512 * nb : 512 * (nb + 1)],
                    in_=o[:],
                )

        # One-stage software pipeline: the second matmul phase of tile s-1 is
        # emitted after the energy matmuls of tile s, so the PE never waits on
        # the softmax/transpose latency between the two phases of one tile.
        staged = []

        def flush_one():
            bb, ii, ss, ptt, xbs_, x32s_ = staged.pop(0)
            emit_mm2_epi(bb, ii, ss, ptt, xbs_, x32s_)

        for b in range(bs):
            x32s, xbs, xt, qt = emit_loads(b)
            for i in range(nct):
                s, pt = emit_mm1_softmax(i, xt, qt)
                staged.append((b, i, s, pt, xbs, x32s))
                if len(staged) >= 2:
                    flush_one()
        while staged:
            flush_one()

    nc.compile()
    return nc


_NC_CACHE = {}


def _get_nc(key=(BS, C, N)):
    if key not in _NC_CACHE:
        _NC_CACHE[key] = build_nc(*key)
    return _NC_CACHE[key]


def _run(x, x_RGB, delta, trace=False):
    x = np.ascontiguousarray(np.asarray(x, dtype=np.float32)).reshape(B, C, N)
    xr = np.ascontiguousarray(np.asarray(x_RGB, dtype=np.float32)).reshape(B, C, N)
    d = np.asarray(delta, dtype=np.float32).reshape(-1)[0]
    d_b = np.full((128, 1), d, dtype=np.float32)

    nc = _get_nc()
    in_maps = []
    for cid in range(N_CORES):
        sl = slice(cid * BS, (cid + 1) * BS)
        in_maps.append(
            {
                "x": np.ascontiguousarray(x[sl]),
                "x_RGB": np.ascontiguousarray(xr[sl]),
                "delta": d_b,
            }
        )
    res = run_bass_kernel_spmd(
        nc, in_maps, core_ids=list(range(N_CORES)), trace=trace
    )
    out = np.concatenate([r["out"] for r in res.results], axis=0)
    return out.reshape(B, C, H, W).astype(np.float32), res


def kernel(x, x_RGB, delta):
    out, _ = _run(x, x_RGB, delta, trace=False)
    return out
